# revision 9
# baseline (speedup 1.0000x reference)
"""L1-loss kernel for Trainium2: mean over rows of sum(|out - target|).

Data-parallel over 8 NeuronCores: each core streams its row-shard of
`out` and `target` from HBM and produces per-partition partial sums of
|out - target|; the host sums the partials and divides by the global
row count.

Per core the shard is repacked host-side into [NT, 128, 2*FREE] tiles
whose partition rows hold the `out` chunk followed by the `target`
chunk. One DMA then feeds both operands of the subtract, which halves
the DMA count and keeps each compute instruction to a single upstream
semaphore. Per tile: DVE subtract into a scratch tile, then ACT Abs
with free-dim accumulation into a [128, NT] accumulator column — the
two compute engines each make one pass, well under the ~360 GB/s DMA
stream that bounds the kernel (memory-roofline workload).
"""

from contextlib import ExitStack

import numpy as np

import concourse.bass as bass
import concourse.bacc as bacc
import concourse.tile as tile
from concourse import mybir
from concourse.bass_utils import run_bass_kernel_spmd

N_VEH = 8388608
N_FEAT = 8
N_CORES = 8
ROWS_PER_CORE = N_VEH // N_CORES            # 1048576
ELEMS_PER_CORE = ROWS_PER_CORE * N_FEAT     # 8388608
P = 128
FREE = 2048
NT = ELEMS_PER_CORE // (P * FREE)           # 32 tiles; fused tile = [128, 4096] f32 (2 MiB)


def _build_nc() -> bass.Bass:
    # Bacc (not raw Bass): its compile() pass allocates registers and splits
    # multi-sem waits into EventSemaphore instructions — TRN2 instructions
    # fit only one wait.
    nc = bacc.Bacc()
    ot_ext = nc.declare_dram_parameter(
        "ot", [NT, P, 2 * FREE], mybir.dt.float32, isOutput=False
    )
    partials = nc.declare_dram_parameter(
        "partials", [P, NT], mybir.dt.float32, isOutput=True
    )

    with tile.TileContext(nc) as tc, ExitStack() as ctx:
        x_pool = ctx.enter_context(tc.tile_pool(name="x", bufs=6))
        d_pool = ctx.enter_context(tc.tile_pool(name="d", bufs=2))
        acc_pool = ctx.enter_context(tc.tile_pool(name="acc", bufs=1))
        acc = acc_pool.tile([P, NT], mybir.dt.float32)
        for i in range(NT):
            x = x_pool.tile([P, 2 * FREE], mybir.dt.float32)
            nc.sync.dma_start(x[:], ot_ext[i])
            d = d_pool.tile([P, FREE], mybir.dt.float32)
            nc.vector.tensor_tensor(
                out=d[:], in0=x[:, :FREE], in1=x[:, FREE:],
                op=mybir.AluOpType.subtract,
            )
            nc.scalar.activation(
                out=d[:], in_=d[:],
                func=mybir.ActivationFunctionType.Abs,
                accum_out=acc[:, i : i + 1],
            )
        nc.sync.dma_start(partials[:], acc[:])
    # The PJRT exec path serializes the module as-is; finalize() here runs
    # Bacc.compile() (register allocation + the wait-splitting pass).
    nc.finalize()
    return nc


def _pack(out: np.ndarray, target: np.ndarray) -> list[dict[str, np.ndarray]]:
    """Interleave out/target per partition row: core shard -> [NT, P, 2*FREE]."""
    in_maps = []
    for c in range(N_CORES):
        sl = slice(c * ROWS_PER_CORE, (c + 1) * ROWS_PER_CORE)
        ot = np.empty((NT, P, 2 * FREE), dtype=np.float32)
        ot[:, :, :FREE] = out[sl].reshape(NT, P, FREE)
        ot[:, :, FREE:] = target[sl].reshape(NT, P, FREE)
        in_maps.append({"ot": ot})
    return in_maps


def _run(nc: bass.Bass, out: np.ndarray, target: np.ndarray, **kwargs):
    return run_bass_kernel_spmd(nc, _pack(out, target), list(range(N_CORES)), **kwargs)


def kernel(out: np.ndarray, target: np.ndarray, x: np.ndarray | None = None) -> np.ndarray:
    out = np.ascontiguousarray(np.asarray(out, dtype=np.float32))
    target = np.ascontiguousarray(np.asarray(target, dtype=np.float32))
    res = _run(_build_nc(), out, target)
    total = sum(r["partials"].astype(np.float64).sum() for r in res.results)
    return np.float32(total / N_VEH)
